# revision 6
# baseline (speedup 1.0000x reference)
"""Sigmoid-attention Bass kernel for TRN2, 8 NeuronCores (batch-parallel).

Problem (per batch element b, one per core):
    S = Q^T K            [2048, 2048]   (contract over d=128)
    P = sigmoid(S/sqrt(128))
    O = V P              [128, 2048]

The kernel is ScalarE-bound: 4.19M sigmoid elements/core at 1 elem/cyc/lane
@1.2GHz = 27.3us streaming + ~150ns/call overhead -> ~32.2us for 32
[128,1024] ACTIVATEs.  Everything else is scheduled around keeping that
stream gapless:

  - All tensors bf16 on device (host casts fp32->bf16; rel-err budget 2e-2,
    measured contribution ~4e-3).  Halves HBM traffic (1.5MB in/0.25MB out
    per core) and enables FWL fast weight loads on the PE.
  - V is pre-transposed per 128-block on the HOST (VT[t, 128j+v] =
    V[v, 128j+t]) so the device does zero transposes: PE only runs the
    S- and O-matmuls (4x512-col bf16 per tile = ~880ns < 1005ns ACT cadence).
  - All inputs are packed into ONE DRAM tensor in consumption order: DMA
    cost is packet-bound (128 partition-row packets x ~160ns / 16 engines
    per transfer, ~1.3us floor), so the first transfer carries everything
    the first two pipeline iterations need.
  - ScalarE runs ONLY the sigmoid stream (the act-table load auto-inserts
    ahead of it and executes during the DMA fill).  All input DMA triggers
    live on the sync queue, sized/ordered so S(0) is ready ~1us after the
    first transfer lands; all PSUM->SBUF drains are VectorE copies (no
    scalar.copy -> no second act-table set -> no mid-kernel table switch).
  - A short burst of 256-col bf16 junk matmuls bridges PE from body start
    to the first K/Q arrival so the HAM clock-gate sees sustained activity
    and unthrottles to 2.4GHz early.
  - O-matmuls are deferred 3 iterations behind the sigmoids so the PE's
    in-order stream never parks on the current sigmoid; PSUM: S double-
    buffered (4 banks) + O double-buffered (4 banks).
"""

import numpy as np
import ml_dtypes

import concourse.bass as bass
import concourse.tile as tile
from concourse import bacc, mybir
from concourse.bass_utils import run_bass_kernel_spmd

B, D, N = 8, 128, 2048
NT = N // 128            # 16 n-tiles of 128
MH = 2                   # m halves
MW = N // MH             # 1024 columns per half
CH = MW // 512           # 512-wide matmul chunks per half
SCALE = float(1.0 / np.sqrt(128.0))
F32 = mybir.dt.float32
BF16 = mybir.dt.bfloat16
NPBF16 = ml_dtypes.bfloat16
SIG = mybir.ActivationFunctionType.Sigmoid

JUNK_N = 22              # HAM warmup matmuls (128 cols bf16 each)

_CACHED_NC = None


XW = 3 * N  # packed input width (7 transfers, consumption order)
# Packed SBUF layout (bf16 cols):
#   X1a @    0: K[0:512]    | Q[0:128]     (640)  -> gates S(0)c0
#   X1b @  640: K[512:1024] | Q[128:256]   (640)  -> gates S(0)c1, S(1)
#   X2a @ 1280: VT[0:256]   | Q[256:512]   (512)  -> O(0..1), S(2..3)
#   X2b @ 1792: Q[512:2048]               (1536)  -> S(4..15)
#   X3  @ 3328: VT[256:1024]               (768)  -> O(2..7)
#   X4  @ 4096: K[1024:2048]              (1024)  -> S(16..31)
#   X5  @ 5120: VT[1024:2048]             (1024)  -> O(8..15)


def _q_col(c):
    """Packed-layout column of Q column c."""
    if c < 128:
        return 512 + c
    if c < 256:
        return 1024 + c
    return 1280 + c


def _vt_col(c):
    """Packed-layout column of VT column c."""
    if c < 256:
        return 1280 + c
    if c < 1024:
        return 3072 + c
    return 4096 + c


def _k_col(c):
    """Packed-layout column of K column c."""
    if c < 512:
        return c
    if c < 1024:
        return 128 + c
    return 3072 + c


def build_nc():
    nc = bacc.Bacc("TRN2", target_bir_lowering=False, debug=False, num_devices=B,
                   enable_asserts=False)
    # One DRAM tensor PER TRANSFER, each fully contiguous, so the DMA
    # engines stream sequential HBM rows (a column-slice of one wide tensor
    # reads only a fraction of each DRAM row and wastes row activations).
    # The first transfer is deliberately SMALL (just what S(0)c0 needs):
    # a [128, w] transfer costs ~128 packets of (~115ns + bytes) over 16
    # engines, so shrinking w shrinks the critical-path landing time.
    x1a = nc.dram_tensor("X1A", [D, 640], BF16, kind="ExternalInput").ap()
    x1b = nc.dram_tensor("X1B", [D, 640], BF16, kind="ExternalInput").ap()
    x2a = nc.dram_tensor("X2A", [D, 512], BF16, kind="ExternalInput").ap()
    x2b = nc.dram_tensor("X2B", [D, 1536], BF16, kind="ExternalInput").ap()
    x3 = nc.dram_tensor("X3", [D, 768], BF16, kind="ExternalInput").ap()
    x4 = nc.dram_tensor("X4", [D, 1024], BF16, kind="ExternalInput").ap()
    x5 = nc.dram_tensor("X5", [D, 1024], BF16, kind="ExternalInput").ap()
    out_ext = nc.dram_tensor("OUT", [D, N], BF16, kind="ExternalOutput").ap()

    with tile.TileContext(nc) as tc:
        with (
            tc.tile_pool(name="sb", bufs=1) as sb,
            tc.tile_pool(name="pp", bufs=6) as pp,
            tc.tile_pool(name="ob", bufs=2) as ob,
            tc.tile_pool(name="ps", bufs=2, space="PSUM") as ps,
            # O accumulators as per-512-chunk tiles (1 bank each) so the
            # drain of chunk c depends only on chunk c's last matmul.
            tc.tile_pool(name="po", bufs=4, space="PSUM") as po,
        ):
            x_sb = sb.tile([D, XW], BF16, tag="x", name="x_sb")

            # Input DMA schedule (sync HWDGE queue, in issue order).  DMA
            # cost is packet-bound: ~160-220ns per partition-row packet, 128
            # packets per transfer over 16 engines -> ~1.3us floor per
            # transfer regardless of bytes.  The host packs all inputs into
            # one DRAM tensor in CONSUMPTION order, so the critical prefix
            # (K half 0 + Q tile 0-1 + VT block 0-1, everything sig(0)/O(0)
            # need) is ONE 128-packet transfer.  Later needs: Q[256:] by
            # sig(2) ~11.5us, VT[256:1024] by O(2) ~12us, K h1 by sig(16)
            # ~26us, VT[1024:] by O(8) ~18us.
            nc.sync.dma_start(out=x_sb[:, 0:640], in_=x1a[:])
            nc.sync.dma_start(out=x_sb[:, 640:1280], in_=x1b[:])
            nc.sync.dma_start(out=x_sb[:, 1280:1792], in_=x2a[:])
            nc.sync.dma_start(out=x_sb[:, 1792:3328], in_=x2b[:])
            nc.sync.dma_start(out=x_sb[:, 3328:4096], in_=x3[:])
            nc.sync.dma_start(out=x_sb[:, 4096:5120], in_=x4[:])
            nc.sync.dma_start(out=x_sb[:, 5120:XW], in_=x5[:])

            junk = sb.tile([D, D], BF16, tag="junk", name="junk")
            nc.gpsimd.memset(junk[:], 0.0)

            # HAM warmup: keep the PE busy from body start until the first
            # real S-matmul's inputs land, so the clock-gate opens to 2.4GHz.
            wps = po.tile([D, 512], F32, tag="o", name="warm_ps")
            for w in range(JUNK_N):
                nc.tensor.matmul(wps[:, 0:D], lhsT=junk[:],
                                 rhs=junk[:], start=True, stop=True)

            # O-matmuls run `DEFER` iterations behind the sigmoids so the
            # PE's in-order stream never waits on the current sigmoid.
            pending = []
            DEFER = 3

            def flush(p):
                o_ps, p_t, n, h = p
                vtc = _vt_col(n * D)
                for c in range(CH):
                    nc.tensor.matmul(
                        o_ps[c][:],
                        lhsT=x_sb[:, vtc:vtc + D],
                        rhs=p_t[:, bass.ts(c, 512)],
                        start=(n == 0),
                        stop=(n == NT - 1),
                    )
                if n == NT - 1:
                    # Drain this half: copy PSUM->SBUF (bf16) then DMA out
                    # via the sync queue (idle by now).  h0 (mid-kernel)
                    # stays entirely off ScalarE; for h1 (after the last
                    # sigmoid, ACT idle) the two 512-chunks drain in
                    # parallel on Vector and Scalar with separate DMA
                    # triggers so the first transfer starts ~1us earlier.
                    o_out = ob.tile([D, MW], BF16, tag="oo", name=f"o_out{h}")
                    if h == 0:
                        for c in range(CH):
                            nc.vector.tensor_copy(o_out[:, bass.ts(c, 512)],
                                                  o_ps[c][:])
                        nc.sync.dma_start(out=out_ext[:, 0:MW], in_=o_out[:])
                    else:
                        # c0's O-matmul completes DURING the last sigmoid
                        # half, so ScalarE (free right at stream end) copies
                        # it immediately; VectorE takes c1 the moment its
                        # O-matmul lands.  Each engine then issues its own
                        # chunk's DMA trigger from its OWN queue, so the two
                        # ~620ns trigger issues run in parallel instead of
                        # serializing on the sync queue.
                        nc.scalar.copy(o_out[:, 0:512], o_ps[0][:])
                        nc.scalar.dma_start(out=out_ext[:, MW:MW + 512],
                                            in_=o_out[:, 0:512])
                        nc.vector.tensor_copy(o_out[:, 512:MW], o_ps[1][:])
                        nc.sync.dma_start(out=out_ext[:, MW + 512:N],
                                          in_=o_out[:, 512:MW])

            for h in range(MH):
                o_ps = [po.tile([D, 512], F32, tag="o", name=f"o_ps{h}_{c}")
                        for c in range(CH)]
                for n in range(NT):
                    s_ps = ps.tile([D, MW], F32, tag="s", name=f"s{h}_{n}")
                    qc = _q_col(n * D)
                    for c in range(CH):
                        kc = _k_col(h * MW + c * 512)
                        nc.tensor.matmul(
                            s_ps[:, bass.ts(c, 512)],
                            lhsT=x_sb[:, qc:qc + D],
                            rhs=x_sb[:, kc:kc + 512],
                            start=True,
                            stop=True,
                        )
                    p_t = pp.tile([D, MW], BF16, tag="p", name=f"p{h}_{n}")
                    first = (h == 0 and n == 0)
                    last = (h == MH - 1 and n == NT - 1)
                    if first or last:
                        # Split the first sigmoid (starts as soon as S(0)'s
                        # first 512-chunk lands) and the final one (so the
                        # last O-matmul and drain start earlier); each split
                        # costs one extra ACTIVATE overhead (~140ns).
                        for c in range(CH):
                            nc.scalar.activation(p_t[:, bass.ts(c, 512)],
                                                 s_ps[:, bass.ts(c, 512)],
                                                 SIG, scale=SCALE)
                    else:
                        nc.scalar.activation(p_t[:], s_ps[:], SIG, scale=SCALE)
                    pending.append((o_ps, p_t, n, h))
                    if len(pending) > DEFER:
                        flush(pending.pop(0))
            while pending:
                flush(pending.pop(0))

    nc.compile()
    return nc


def prepare_in_maps(inputs):
    """Host-side prep: cast to bf16, block-transpose V, pack in consumption
    order into one DRAM tensor.

    VT[b, t, 128j+v] = V[b, v, 128j+t]  so that VT[:, 128n:128n+128] is
    directly the lhsT (stationary operand) of the O-matmul for n-block n.
    Packed layout (bf16 cols): K[:, 0:1024] | Q[:, 0:256] | VT[:, 0:256] |
    Q[:, 256:2048] | VT[:, 256:1024] | K[:, 1024:2048] | VT[:, 1024:2048].
    """
    Q = np.asarray(inputs["Q"], dtype=np.float32)
    K = np.asarray(inputs["K"], dtype=np.float32)
    V = np.asarray(inputs["V"], dtype=np.float32)
    assert Q.shape == (B, D, N), Q.shape
    Qb = Q.astype(NPBF16)
    Kb = K.astype(NPBF16)
    VT = np.ascontiguousarray(
        V.reshape(B, D, NT, D).transpose(0, 3, 2, 1)
    ).reshape(B, D, N).astype(NPBF16)
    X1A = np.ascontiguousarray(np.concatenate(
        [Kb[:, :, 0:512], Qb[:, :, 0:128]], axis=2))
    X1B = np.ascontiguousarray(np.concatenate(
        [Kb[:, :, 512:1024], Qb[:, :, 128:256]], axis=2))
    X2A = np.ascontiguousarray(np.concatenate(
        [VT[:, :, 0:256], Qb[:, :, 256:512]], axis=2))
    X2B = np.ascontiguousarray(Qb[:, :, 512:N])
    X3 = np.ascontiguousarray(VT[:, :, 256:1024])
    X4 = np.ascontiguousarray(Kb[:, :, 1024:N])
    X5 = np.ascontiguousarray(VT[:, :, 1024:N])
    return [{"X1A": X1A[i], "X1B": X1B[i], "X2A": X2A[i], "X2B": X2B[i],
             "X3": X3[i], "X4": X4[i], "X5": X5[i]}
            for i in range(B)]


def kernel(**inputs):
    global _CACHED_NC
    in_maps = prepare_in_maps(inputs)

    if _CACHED_NC is None:
        _CACHED_NC = build_nc()
    nc = _CACHED_NC

    res = run_bass_kernel_spmd(nc, in_maps, core_ids=list(range(B)))
    out = np.stack([res.results[i]["OUT"] for i in range(B)], axis=0)
    return out.astype(np.float32)


if __name__ == "__main__":
    rng = np.random.default_rng(0)
    ins = {
        "Q": rng.standard_normal((B, D, N)).astype(np.float32),
        "K": rng.standard_normal((B, D, N)).astype(np.float32),
        "V": rng.standard_normal((B, D, N)).astype(np.float32),
    }
    out = kernel(**ins)
    print("kernel output", out.shape, out.dtype)



# revision 8
# speedup vs baseline: 1.0119x; 1.0119x over previous
"""Sigmoid-attention Bass kernel for TRN2, 8 NeuronCores (batch-parallel).

Problem (per batch element b, one per core):
    S = Q^T K            [2048, 2048]   (contract over d=128)
    P = sigmoid(S/sqrt(128))
    O = V P              [128, 2048]

The kernel is ScalarE-bound: 4.19M sigmoid elements/core at 1 elem/cyc/lane
@1.2GHz = 27.3us streaming + ~150ns/call overhead -> ~32.2us for 32
[128,1024] ACTIVATEs.  Everything else is scheduled around keeping that
stream gapless:

  - All tensors bf16 on device (host casts fp32->bf16; rel-err budget 2e-2,
    measured contribution ~4e-3).  Halves HBM traffic (1.5MB in/0.25MB out
    per core) and enables FWL fast weight loads on the PE.
  - V is pre-transposed per 128-block on the HOST (VT[t, 128j+v] =
    V[v, 128j+t]) so the device does zero transposes: PE only runs the
    S- and O-matmuls (4x512-col bf16 per tile = ~880ns < 1005ns ACT cadence).
  - All inputs are packed into ONE DRAM tensor in consumption order: DMA
    cost is packet-bound (128 partition-row packets x ~160ns / 16 engines
    per transfer, ~1.3us floor), so the first transfer carries everything
    the first two pipeline iterations need.
  - ScalarE runs ONLY the sigmoid stream (the act-table load auto-inserts
    ahead of it and executes during the DMA fill).  All input DMA triggers
    live on the sync queue, sized/ordered so S(0) is ready ~1us after the
    first transfer lands; all PSUM->SBUF drains are VectorE copies (no
    scalar.copy -> no second act-table set -> no mid-kernel table switch).
  - A short burst of 256-col bf16 junk matmuls bridges PE from body start
    to the first K/Q arrival so the HAM clock-gate sees sustained activity
    and unthrottles to 2.4GHz early.
  - O-matmuls are deferred 3 iterations behind the sigmoids so the PE's
    in-order stream never parks on the current sigmoid; PSUM: S double-
    buffered (4 banks) + O double-buffered (4 banks).
"""

import numpy as np
import ml_dtypes

import concourse.bass as bass
import concourse.tile as tile
from concourse import bacc, mybir
from concourse.bass_utils import run_bass_kernel_spmd

B, D, N = 8, 128, 2048
NT = N // 128            # 16 n-tiles of 128
MH = 2                   # m halves
MW = N // MH             # 1024 columns per half
CH = MW // 512           # 512-wide matmul chunks per half
SCALE = float(1.0 / np.sqrt(128.0))
F32 = mybir.dt.float32
BF16 = mybir.dt.bfloat16
NPBF16 = ml_dtypes.bfloat16
SIG = mybir.ActivationFunctionType.Sigmoid

JUNK_N = 15              # HAM warmup matmuls (128 cols bf16 each)

_CACHED_NC = None


XW = 3 * N  # packed input width (7 transfers, consumption order)
# Packed SBUF layout (bf16 cols):
#   X1a @    0: K[0:512]    | Q[0:128]     (640)  -> gates S(0)c0
#   X1b @  640: K[512:1024] | Q[128:256]   (640)  -> gates S(0)c1, S(1)
#   X2a @ 1280: VT[0:256]   | Q[256:512]   (512)  -> O(0..1), S(2..3)
#   X2b @ 1792: Q[512:2048]               (1536)  -> S(4..15)
#   X3  @ 3328: VT[256:1024]               (768)  -> O(2..7)
#   X4  @ 4096: K[1024:2048]              (1024)  -> S(16..31)
#   X5  @ 5120: VT[1024:2048]             (1024)  -> O(8..15)


def _q_col(c):
    """Packed-layout column of Q column c."""
    if c < 128:
        return 512 + c
    if c < 256:
        return 1024 + c
    return 1280 + c


def _vt_col(c):
    """Packed-layout column of VT column c."""
    if c < 256:
        return 1280 + c
    if c < 1024:
        return 3072 + c
    return 4096 + c


def _k_col(c):
    """Packed-layout column of K column c."""
    if c < 512:
        return c
    if c < 1024:
        return 128 + c
    return 3072 + c


def build_nc():
    nc = bacc.Bacc("TRN2", target_bir_lowering=False, debug=False, num_devices=B,
                   enable_asserts=False)
    # One DRAM tensor PER TRANSFER, each fully contiguous, so the DMA
    # engines stream sequential HBM rows (a column-slice of one wide tensor
    # reads only a fraction of each DRAM row and wastes row activations).
    # The first transfer is deliberately SMALL (just what S(0)c0 needs):
    # a [128, w] transfer costs ~128 packets of (~115ns + bytes) over 16
    # engines, so shrinking w shrinks the critical-path landing time.
    x1a = nc.dram_tensor("X1A", [D, 640], BF16, kind="ExternalInput").ap()
    x1b = nc.dram_tensor("X1B", [D, 640], BF16, kind="ExternalInput").ap()
    x2a = nc.dram_tensor("X2A", [D, 512], BF16, kind="ExternalInput").ap()
    x2b = nc.dram_tensor("X2B", [D, 1536], BF16, kind="ExternalInput").ap()
    x3 = nc.dram_tensor("X3", [D, 768], BF16, kind="ExternalInput").ap()
    x4 = nc.dram_tensor("X4", [D, 1024], BF16, kind="ExternalInput").ap()
    x5 = nc.dram_tensor("X5", [D, 1024], BF16, kind="ExternalInput").ap()
    out_ext = nc.dram_tensor("OUT", [D, N], BF16, kind="ExternalOutput").ap()

    with tile.TileContext(nc) as tc:
        with (
            tc.tile_pool(name="sb", bufs=1) as sb,
            tc.tile_pool(name="pp", bufs=6) as pp,
            tc.tile_pool(name="ob", bufs=2) as ob,
            tc.tile_pool(name="ps", bufs=2, space="PSUM") as ps,
            # O accumulators as per-512-chunk tiles (1 bank each) so the
            # drain of chunk c depends only on chunk c's last matmul.
            tc.tile_pool(name="po", bufs=4, space="PSUM") as po,
        ):
            x_sb = sb.tile([D, XW], BF16, tag="x", name="x_sb")

            # Input DMA schedule (sync HWDGE queue, in issue order).  DMA
            # cost is packet-bound: ~160-220ns per partition-row packet, 128
            # packets per transfer over 16 engines -> ~1.3us floor per
            # transfer regardless of bytes.  The host packs all inputs into
            # one DRAM tensor in CONSUMPTION order, so the critical prefix
            # (K half 0 + Q tile 0-1 + VT block 0-1, everything sig(0)/O(0)
            # need) is ONE 128-packet transfer.  Later needs: Q[256:] by
            # sig(2) ~11.5us, VT[256:1024] by O(2) ~12us, K h1 by sig(16)
            # ~26us, VT[1024:] by O(8) ~18us.
            nc.sync.dma_start(out=x_sb[:, 0:640], in_=x1a[:])
            nc.sync.dma_start(out=x_sb[:, 640:1280], in_=x1b[:])
            nc.sync.dma_start(out=x_sb[:, 1280:1792], in_=x2a[:])
            nc.sync.dma_start(out=x_sb[:, 1792:3328], in_=x2b[:])
            nc.sync.dma_start(out=x_sb[:, 3328:4096], in_=x3[:])
            nc.sync.dma_start(out=x_sb[:, 4096:5120], in_=x4[:])
            nc.sync.dma_start(out=x_sb[:, 5120:XW], in_=x5[:])

            junk = sb.tile([D, D], BF16, tag="junk", name="junk")
            nc.gpsimd.memset(junk[:], 0.0)

            # HAM warmup: keep the PE busy from body start until the first
            # real S-matmul's inputs land, so the clock-gate opens to 2.4GHz.
            wps = po.tile([D, 512], F32, tag="o", name="warm_ps")
            for w in range(JUNK_N):
                nc.tensor.matmul(wps[:, 0:D], lhsT=junk[:],
                                 rhs=junk[:], start=True, stop=True)

            # O-matmuls run `DEFER` iterations behind the sigmoids so the
            # PE's in-order stream never waits on the current sigmoid.
            pending = []
            DEFER = 3

            def flush(p):
                o_ps, p_t, n, h = p
                vtc = _vt_col(n * D)
                for c in range(CH):
                    nc.tensor.matmul(
                        o_ps[c][:],
                        lhsT=x_sb[:, vtc:vtc + D],
                        rhs=p_t[:, bass.ts(c, 512)],
                        start=(n == 0),
                        stop=(n == NT - 1),
                    )
                if n == NT - 1:
                    # Drain this half: copy PSUM->SBUF (bf16) then DMA out
                    # via the sync queue (idle by now).  h0 (mid-kernel)
                    # stays entirely off ScalarE; for h1 (after the last
                    # sigmoid, ACT idle) the two 512-chunks drain in
                    # parallel on Vector and Scalar with separate DMA
                    # triggers so the first transfer starts ~1us earlier.
                    o_out = ob.tile([D, MW], BF16, tag="oo", name=f"o_out{h}")
                    if h == 0:
                        for c in range(CH):
                            nc.vector.tensor_copy(o_out[:, bass.ts(c, 512)],
                                                  o_ps[c][:])
                        nc.sync.dma_start(out=out_ext[:, 0:MW], in_=o_out[:])
                    else:
                        # c0's O-matmul completes DURING the last sigmoid
                        # half, so ScalarE (free right at stream end) copies
                        # it immediately; VectorE takes c1 the moment its
                        # O-matmul lands.  Each engine then issues its own
                        # chunk's DMA trigger from its OWN queue, so the two
                        # ~620ns trigger issues run in parallel instead of
                        # serializing on the sync queue.
                        nc.scalar.copy(o_out[:, 0:512], o_ps[0][:])
                        nc.scalar.dma_start(out=out_ext[:, MW:MW + 512],
                                            in_=o_out[:, 0:512])
                        nc.vector.tensor_copy(o_out[:, 512:MW], o_ps[1][:])
                        nc.sync.dma_start(out=out_ext[:, MW + 512:N],
                                          in_=o_out[:, 512:MW])

            for h in range(MH):
                o_ps = [po.tile([D, 512], F32, tag="o", name=f"o_ps{h}_{c}")
                        for c in range(CH)]
                for n in range(NT):
                    qc = _q_col(n * D)
                    p_t = pp.tile([D, MW], BF16, tag="p", name=f"p{h}_{n}")
                    first = (h == 0 and n == 0)
                    last = (h == MH - 1 and n == NT - 1)
                    if first or last:
                        # Split the first sigmoid (starts as soon as S(0)'s
                        # first 512-chunk lands) and the final one (so the
                        # last O-matmul and drain start earlier).  Use two
                        # SEPARATE 512-wide PSUM tiles and interleave
                        # matmul/activation per chunk so dependency tracking
                        # is exact: sigmoid(c0) must not wait on the c1
                        # matmul (whose input lands in a later DMA).
                        for c in range(CH):
                            s_c = ps.tile([D, 512], F32, tag="s",
                                          name=f"s{h}_{n}_{c}")
                            kc = _k_col(h * MW + c * 512)
                            nc.tensor.matmul(
                                s_c[:],
                                lhsT=x_sb[:, qc:qc + D],
                                rhs=x_sb[:, kc:kc + 512],
                                start=True,
                                stop=True,
                            )
                            nc.scalar.activation(p_t[:, bass.ts(c, 512)],
                                                 s_c[:], SIG, scale=SCALE)
                    else:
                        s_ps = ps.tile([D, MW], F32, tag="s", name=f"s{h}_{n}")
                        for c in range(CH):
                            kc = _k_col(h * MW + c * 512)
                            nc.tensor.matmul(
                                s_ps[:, bass.ts(c, 512)],
                                lhsT=x_sb[:, qc:qc + D],
                                rhs=x_sb[:, kc:kc + 512],
                                start=True,
                                stop=True,
                            )
                        nc.scalar.activation(p_t[:], s_ps[:], SIG, scale=SCALE)
                    pending.append((o_ps, p_t, n, h))
                    if len(pending) > DEFER:
                        flush(pending.pop(0))
            while pending:
                flush(pending.pop(0))

    nc.compile()
    return nc


def prepare_in_maps(inputs):
    """Host-side prep: cast to bf16, block-transpose V, pack in consumption
    order into one DRAM tensor.

    VT[b, t, 128j+v] = V[b, v, 128j+t]  so that VT[:, 128n:128n+128] is
    directly the lhsT (stationary operand) of the O-matmul for n-block n.
    Packed layout (bf16 cols): K[:, 0:1024] | Q[:, 0:256] | VT[:, 0:256] |
    Q[:, 256:2048] | VT[:, 256:1024] | K[:, 1024:2048] | VT[:, 1024:2048].
    """
    Q = np.asarray(inputs["Q"], dtype=np.float32)
    K = np.asarray(inputs["K"], dtype=np.float32)
    V = np.asarray(inputs["V"], dtype=np.float32)
    assert Q.shape == (B, D, N), Q.shape
    Qb = Q.astype(NPBF16)
    Kb = K.astype(NPBF16)
    VT = np.ascontiguousarray(
        V.reshape(B, D, NT, D).transpose(0, 3, 2, 1)
    ).reshape(B, D, N).astype(NPBF16)
    X1A = np.ascontiguousarray(np.concatenate(
        [Kb[:, :, 0:512], Qb[:, :, 0:128]], axis=2))
    X1B = np.ascontiguousarray(np.concatenate(
        [Kb[:, :, 512:1024], Qb[:, :, 128:256]], axis=2))
    X2A = np.ascontiguousarray(np.concatenate(
        [VT[:, :, 0:256], Qb[:, :, 256:512]], axis=2))
    X2B = np.ascontiguousarray(Qb[:, :, 512:N])
    X3 = np.ascontiguousarray(VT[:, :, 256:1024])
    X4 = np.ascontiguousarray(Kb[:, :, 1024:N])
    X5 = np.ascontiguousarray(VT[:, :, 1024:N])
    return [{"X1A": X1A[i], "X1B": X1B[i], "X2A": X2A[i], "X2B": X2B[i],
             "X3": X3[i], "X4": X4[i], "X5": X5[i]}
            for i in range(B)]


def kernel(**inputs):
    global _CACHED_NC
    in_maps = prepare_in_maps(inputs)

    if _CACHED_NC is None:
        _CACHED_NC = build_nc()
    nc = _CACHED_NC

    res = run_bass_kernel_spmd(nc, in_maps, core_ids=list(range(B)))
    out = np.stack([res.results[i]["OUT"] for i in range(B)], axis=0)
    return out.astype(np.float32)


if __name__ == "__main__":
    rng = np.random.default_rng(0)
    ins = {
        "Q": rng.standard_normal((B, D, N)).astype(np.float32),
        "K": rng.standard_normal((B, D, N)).astype(np.float32),
        "V": rng.standard_normal((B, D, N)).astype(np.float32),
    }
    out = kernel(**ins)
    print("kernel output", out.shape, out.dtype)

